# revision 34
# baseline (speedup 1.0000x reference)
"""Trainium2 Bass kernel for nn_ColorHistograms (histogram -> banded sims -> windowed fc).

Sharding: data-parallel, one video (T=1024 frames) per NeuronCore, 8 cores, SPMD.

Per-core pipeline:
  1. 512-bin color histogram per frame, factored 512 = 32 x 16:
     DVE builds one-hots (is_equal vs iota consts, bf16 2x mode); PE contracts
     128-pixel chunks (oh16^T @ oh32) into PSUM, 4 frames col-tiled concurrently.
  2. PE transposes + compaction copies -> X2[32*slab + v32, 16*F + v16] bf16
     (exact integer counts); slab-mirror DMAs so each 32-row slab has all frames.
  3. Banded sims (|t-s| <= 50 needs only adjacent 128-blocks): 16 K=32 matmuls
     per block pair accumulate exact integer <h_t, h_s> into PSUM strips.
  4. Strips -> DRAM; skewed-AP DMA (stride 385) gathers windows[t, w] =
     sims[t, t+w-50]. The w=50 column is ||h_t||^2 (diag) for free.
  5. Normalize after the gather: inv = rsqrt(diag) via sqrt + reciprocal +
     one Newton step; winN = win * inv[t] * inv[t+w-50] (zero-padded edges
     reproduce the reference's zero padding).
  6. fc: PE transpose of winN -> matmul with host-pretransposed W_fc^T;
     bias + relu on DVE; DMA out.
"""

import numpy as np

B = 8
T_FULL = 1024
PX = 32 * 32
LOOKUP_WINDOW = 101
OUT_DIM = 128

_CACHE = {}


def build_nc(T=T_FULL, quant_offset=None, phases="all", dve_reps=1, oh16_ls=True):
    # HW DVE float->int32 output conversion rounds to nearest; the sim
    # truncates. Midpoint offsets make every value >=1/128 away from a
    # rounding boundary, so any nearest-mode is exact on HW.
    # quant_offset=(0,0) reproduces the truncating sim semantics.
    from contextlib import ExitStack

    import concourse.bass as bass
    import concourse.mybir as mybir
    from concourse import bacc
    from concourse.masks import make_identity
    from concourse.tile import TileContext

    dt = mybir.dt
    op = mybir.AluOpType
    act = mybir.ActivationFunctionType
    qoff32, qoff64 = (-15.5 / 32.0, -31.5 / 64.0) if quant_offset is None else quant_offset

    NBLK = T // 128
    NFILL = T // 64

    nc = bacc.Bacc()
    frames_d = nc.declare_dram_parameter("frames", [T, PX * 3], dt.int32, isOutput=False)
    wfct_d = nc.declare_dram_parameter("wfct", [LOOKUP_WINDOW, OUT_DIM], dt.float32, isOutput=False)
    bfc_d = nc.declare_dram_parameter("bfc", [1, OUT_DIM], dt.float32, isOutput=False)
    y_d = nc.declare_dram_parameter("y", [T, OUT_DIM], dt.float32, isOutput=True)

    strips_d = nc.dram_tensor("strips", [NBLK, 128, 384], dt.float32)
    invpad_d = nc.dram_tensor("invpad", [1, 64 + T + 64], dt.float32)

    with TileContext(nc) as tc, ExitStack() as ctx:
        singles = ctx.enter_context(tc.tile_pool(name="singles", bufs=1))
        raws = ctx.enter_context(tc.tile_pool(name="raws", bufs=4))
        preps = ctx.enter_context(tc.tile_pool(name="preps", bufs=4))
        ohs = ctx.enter_context(tc.tile_pool(name="ohs", bufs=4))
        hsts = ctx.enter_context(tc.tile_pool(name="hsts", bufs=3))
        strips_p = ctx.enter_context(tc.tile_pool(name="strips_p", bufs=2))
        winp = ctx.enter_context(tc.tile_pool(name="winp", bufs=3))
        allwin = ctx.enter_context(tc.tile_pool(name="allwin", bufs=1))
        smalls = ctx.enter_context(tc.tile_pool(name="smalls", bufs=8))
        outs = ctx.enter_context(tc.tile_pool(name="outs", bufs=3))

        # ---------------- constants ----------------
        iota32 = singles.tile([128, 32, 128], dt.bfloat16)
        nc.gpsimd.iota(iota32, pattern=[[1, 32], [0, 128]], base=0,
                       channel_multiplier=0, allow_small_or_imprecise_dtypes=True)
        iota16 = singles.tile([128, 16, 128], dt.bfloat16)
        nc.gpsimd.iota(iota16, pattern=[[1, 16], [0, 128]], base=0,
                       channel_multiplier=0, allow_small_or_imprecise_dtypes=True)
        iotac16 = singles.tile([128, 8, 8], dt.int16)
        # value = 16 * (8*fh + c) : col base within a local_scatter half
        nc.gpsimd.iota(iotac16, pattern=[[128, 8], [16, 8]], base=0,
                       channel_multiplier=0)
        # dense variant over all 16 frames: value = 16*(8*(f%8) + c)
        iotacf = singles.tile([128, 2, 8, 8], dt.int16)
        nc.gpsimd.iota(iotacf, pattern=[[0, 2], [128, 8], [16, 8]], base=0,
                       channel_multiplier=0)
        ones_ls = singles.tile([128, 128], dt.bfloat16)
        nc.vector.memset(ones_ls, 1.0)
        ident_f = singles.tile([128, 128], dt.float32)
        make_identity(nc, ident_f)
        # parity selectors for the hist evacuation matmuls:
        # bsel[pr][32g + 16h + v16, 16g + v16] = 1 iff h == pr
        bsel = []
        for pr in range(2):
            tmpb = singles.tile([128, 4, 16], dt.int16, tag=f"tmpb{pr}")
            nc.gpsimd.iota(tmpb, pattern=[[-32, 4], [-1, 16]], base=-16 * pr,
                           channel_multiplier=1)
            bs = singles.tile([128, 64], dt.bfloat16, tag=f"bs{pr}")
            nc.vector.tensor_scalar(
                bs, tmpb.rearrange("p g v -> p (g v)"), 0.0, None, op0=op.is_equal)
            bsel.append(bs)
        wfct_sb = singles.tile([LOOKUP_WINDOW, OUT_DIM], dt.float32)
        nc.sync.dma_start(out=wfct_sb, in_=wfct_d[:, :])
        b_rep = singles.tile([128, OUT_DIM], dt.float32)
        nc.sync.dma_start(out=b_rep, in_=bfc_d[:, :].broadcast_to([128, OUT_DIM]))
        zeros128 = singles.tile([128, 128], dt.float32)
        nc.vector.memset(zeros128, 0.0)

        # zero the never-written edges of strips + invpad
        nc.sync.dma_start(out=strips_d[0, :, 0:128], in_=zeros128)
        nc.sync.dma_start(out=strips_d[NBLK - 1, :, 256:384], in_=zeros128)
        nc.sync.dma_start(out=invpad_d[0:1, 0:64], in_=zeros128[0:1, 0:64])
        nc.sync.dma_start(out=invpad_d[0:1, 64 + T:64 + T + 64], in_=zeros128[0:1, 0:64])

        # X2[32*slab + v32, 16*F + v16]  (bin = 16*v32 + v16)
        x2 = singles.tile([128, 16 * T], dt.bfloat16)

        frames_flat = frames_d[:, :].flatten()

        # ---------------- Phase H: histograms ----------------
        # Paired-chunk matmuls: each K-row carries the 2 pixels of chunks
        # (2c2, 2c2+1) via disjoint col-blocks, so one LDW+MM covers 256 px
        # (4 MM + 4 LDW per frame instead of 8+8). Real [16, 32] blocks land
        # at parity-split rows (h = chunk LSB); cross-parity blocks are
        # garbage. PSUM[32g + 16h + v16, 64k + 32pr + v32], 32 frames/bank.
        # Evacuate the whole bank with one dense DVE copy (garbage included),
        # then parity-pure strided LDWs x the constant selector bsel[pr]
        # (zeroes garbage rows, sums the two h-rows) accumulate histT in
        # PSUM; small copies scatter histT blocks into x2.
        hctx = ExitStack()
        ps_h = hctx.enter_context(tc.tile_pool(name="ps_h", bufs=2, space="PSUM"))
        ps_bt = hctx.enter_context(tc.tile_pool(name="ps_bt", bufs=2, space="PSUM"))
        sctx = ExitStack()
        ps_s = sctx.enter_context(tc.tile_pool(name="ps_s", bufs=2, space="PSUM"))
        x2v = x2.rearrange("p (f v) -> p f v", v=16)

        def emit_block_sims(i):
            js = [j for j in (i - 1, i, i + 1) if 0 <= j < NBLK]
            if phases == "diag":
                js = [i]
            # all pairs of block i contract on slab i%4 (mirrors only need to
            # populate 3 of the 4 slabs per block)
            rho = i % 4
            strip_sb = strips_p.tile([128, 384], dt.float32, tag="strip_sb")
            for j in js:
                jj = j - i + 1
                # one PSUM bank per pair: concurrent row-group matmul streams
                # to a shared bank fault the PSUM
                psum_s = ps_s.tile([128, 128], dt.float32, tag="psum_s")
                for m in range(16):
                    nc.tensor.matmul(
                        psum_s,
                        lhsT=x2v[32 * rho:32 * rho + 32, 128 * i:128 * i + 128, m],
                        rhs=x2v[32 * rho:32 * rho + 32, 128 * j:128 * j + 128, m],
                        start=(m == 0), stop=(m == 15),
                        tile_position=(32 * rho, 0),
                    )
                nc.scalar.activation(
                    strip_sb[:, 128 * jj:128 * jj + 128], psum_s, act.Copy,
                )
            jj0, jj1 = js[0] - i + 1, js[-1] - i + 1
            nc.scalar.dma_start(
                out=strips_d[i, :, 128 * jj0:128 * (jj1 + 1)],
                in_=strip_sb[:, 128 * jj0:128 * (jj1 + 1)],
            )

        def make_mirror(fill):
            def run():
                # mirror this fill's columns to the slabs that block
                # b = fill//2 is contracted on ({b-1, b, b+1} % 4) so banded
                # sims for finished 128-frame blocks can run during phase H
                b = fill // 2
                needed = {(b - 1) % 4, b % 4, (b + 1) % 4}
                fcols = x2[:, 1024 * fill:1024 * fill + 1024].rearrange(
                    "p (m q) -> p m q", q=256)
                for kap in range(4):
                    srcv = fcols[32 * kap:32 * kap + 32, :,
                                 64 * kap:64 * kap + 64]
                    for rho in sorted(needed - {kap}):
                        nc.sync.dma_start(
                            out=fcols[32 * rho:32 * rho + 32, :,
                                      64 * kap:64 * kap + 64],
                            in_=srcv)
            return run

        fctx = ExitStack()
        ps_f = fctx.enter_context(tc.tile_pool(name="ps_f", bufs=1, space="PSUM"))
        strips_flat = strips_d[:, :, :].flatten()
        invpad_flat = invpad_d[:, :].flatten()
        wins = {}
        invs = {}

        def emit_wins(i):
            # skewed-AP gather windows[t, w] = sims[t, t+w-50]; col 50 is
            # ||h_t||^2; inv = rsqrt via sqrt + reciprocal + one Newton step
            win = allwin.tile([128, LOOKUP_WINDOW], dt.float32, tag=f"win{i}")
            src = bass.AP(
                tensor=strips_flat.tensor,
                offset=strips_flat.offset + i * 128 * 384 + 78,
                ap=[[385, 128], [1, LOOKUP_WINDOW]],
            )
            nc.gpsimd.dma_start(out=win, in_=src)
            wins[i] = win
            nrm2 = smalls.tile([128, 1], dt.float32, tag=f"nrm2_{i}")
            nc.scalar.activation(nrm2, win[:, 50:51], act.Copy)
            s0 = smalls.tile([128, 1], dt.float32, tag=f"s0_{i}")
            nc.scalar.activation(s0, nrm2, act.Sqrt)
            b0 = smalls.tile([128, 1], dt.float32, tag=f"b0_{i}")
            nc.vector.reciprocal(b0, s0)
            c0 = smalls.tile([128, 1], dt.float32, tag=f"c0_{i}")
            nc.vector.tensor_tensor(c0, b0, b0, op.mult)
            d0 = smalls.tile([128, 1], dt.float32, tag=f"d0_{i}")
            nc.vector.tensor_tensor(d0, c0, nrm2, op.mult)
            e0 = smalls.tile([128, 1], dt.float32, tag=f"e0_{i}")
            nc.vector.tensor_scalar(e0, d0, -0.5, 1.5, op0=op.mult, op1=op.add)
            inv = smalls.tile([128, 1], dt.float32, tag=f"inv_{i}")
            nc.vector.tensor_tensor(inv, b0, e0, op.mult)
            invs[i] = inv
            dst = bass.AP(
                tensor=invpad_flat.tensor,
                offset=invpad_flat.offset + 64 + 128 * i,
                ap=[[1, 128], [1, 1]],
            )
            nc.gpsimd.dma_start(out=dst, in_=inv)

        def emit_fc(i):
            win = wins[i]
            invwin = winp.tile([128, LOOKUP_WINDOW], dt.float32, tag="invwin")
            src = bass.AP(
                tensor=invpad_flat.tensor,
                offset=invpad_flat.offset + 64 + 128 * i - 50,
                ap=[[1, 128], [1, LOOKUP_WINDOW]],
            )
            nc.gpsimd.dma_start(out=invwin, in_=src)
            wn1 = winp.tile([128, LOOKUP_WINDOW], dt.float32, tag="wn1")
            nc.vector.tensor_scalar(wn1, win, invs[i], None, op0=op.mult)
            winN = winp.tile([128, LOOKUP_WINDOW], dt.float32, tag="winN")
            nc.vector.tensor_tensor(winN, wn1, invwin, op.mult)
            ps_w = ps_f.tile([LOOKUP_WINDOW, 128], dt.float32, tag="psw")
            nc.tensor.transpose(ps_w, winN, ident_f)
            wT = winp.tile([LOOKUP_WINDOW, 128], dt.float32, tag="wT")
            nc.scalar.activation(wT, ps_w, act.Copy)
            ps_o = ps_f.tile([128, OUT_DIM], dt.float32, tag="pso")
            nc.tensor.matmul(ps_o, lhsT=wT, rhs=wfct_sb, start=True, stop=True)
            s2 = outs.tile([128, OUT_DIM], dt.float32, tag="s2")
            nc.vector.tensor_tensor(s2, ps_o, b_rep, op.add)
            yt = outs.tile([128, OUT_DIM], dt.float32, tag="yt")
            nc.vector.tensor_scalar_max(yt, s2, 0.0)
            nc.gpsimd.dma_start(out=y_d[128 * i:128 * i + 128, :], in_=yt)

        pending_evac = None
        mirror_q = []
        for fill in range(NFILL):
          for half in range(2):
            psum_h = ps_h.tile([128, 512], dt.float32, tag="ph")
            for ft2 in range(2):
                ft = 2 * half + ft2
                fbase = 64 * fill + 16 * ft
                raw = raws.tile([128, 16, 24], dt.int32, tag="raw")
                # flat = f*3072 + (8p + c)*3 + ch  ->  dest [p, f, (c, ch)]
                src = bass.AP(
                    tensor=frames_flat.tensor,
                    offset=frames_flat.offset + fbase * 3072,
                    ap=[[24, 128], [3072, 16], [1, 24]],
                )
                nc.sync.dma_start(out=raw, in_=src)

                # q = round/trunc(raw * (1/32) + off) ; quant_offset makes
                # either conversion mode exact. One dense quant op on ACT
                # frees the DVE for the one-hots; qg6 halves qg on the DVE
                # (round-nearest: k*0.5 - 0.25 rounds to floor(k/2)).
                q_all = preps.tile([128, 16, 24], dt.int32, tag="qall")
                nc.scalar.activation(q_all, raw, act.Copy,
                                     bias=qoff32, scale=1.0 / 32.0)
                qv = q_all.rearrange("p f (c ch) -> p f c ch", ch=3)
                qg6 = preps.tile([128, 16, 8], dt.int32, tag="qg6")
                nc.vector.tensor_scalar(qg6, qv[:, :, :, 1], 0.5, -0.25,
                                        op0=op.mult, op1=op.add)
                v32b = preps.tile([128, 16, 8], dt.bfloat16, tag="v32b")
                nc.vector.scalar_tensor_tensor(v32b, qv[:, :, :, 0], 4.0, qg6,
                                               op0=op.mult, op1=op.add)
                t1 = preps.tile([128, 16, 8], dt.int32, tag="t1")
                nc.vector.scalar_tensor_tensor(t1, qv[:, :, :, 1], 8.0,
                                               qv[:, :, :, 2],
                                               op0=op.mult, op1=op.add)
                v16b = preps.tile([128, 16, 8], dt.bfloat16, tag="v16b")
                if oh16_ls:
                    v16i = preps.tile([128, 16, 8], dt.int16, tag="v16i")
                    # idx = 16*cc + v16 for local_scatter one-hot build
                    nc.vector.scalar_tensor_tensor(v16i, qg6, -16.0, t1,
                                                   op0=op.mult, op1=op.add)
                    nc.vector.tensor_tensor(
                        v16i, v16i,
                        iotacf.rearrange("p h f c -> p (h f) c"),
                        op.add)
                else:
                    nc.vector.scalar_tensor_tensor(v16b, qg6, -16.0, t1,
                                                   op0=op.mult, op1=op.add)

                # one-hots [128, v, 128] ; px-col cc = 8*fl + c
                # (two halves so frames 0-7's matmuls can start earlier)
                oh32 = ohs.tile([128, 32, 128], dt.bfloat16, tag="oh32")
                v32r = v32b.rearrange("p f c -> p (f c)").unsqueeze(1).broadcast_to([128, 32, 128])
                oh16 = ohs.tile([128, 16, 128], dt.bfloat16, tag="oh16")
                for _rep in range(dve_reps):
                    nc.vector.tensor_tensor(oh32[:, :, 0:64], v32r[:, :, 0:64],
                                            iota32[:, :, 0:64], op.is_equal)
                    nc.vector.tensor_tensor(oh32[:, :, 64:128], v32r[:, :, 64:128],
                                            iota32[:, :, 64:128], op.is_equal)
                if oh16_ls:
                    # oh16[p, 16*cc + v] = 1 at v = v16 (cc-major layout)
                    oh16cm = oh16.rearrange("p v c -> p (v c)")
                    vflat = v16i.rearrange("p f c -> p (f c)")
                    for h in range(2):
                        nc.gpsimd.local_scatter(
                            oh16cm[:, 1024 * h:1024 * h + 1024],
                            ones_ls[:, 64 * h:64 * h + 64],
                            vflat[:, 64 * h:64 * h + 64],
                            channels=128, num_elems=1024, num_idxs=64,
                        )
                else:
                    v16r = v16b.rearrange("p f c -> p (f c)").unsqueeze(1).broadcast_to([128, 16, 128])
                    for _rep in range(dve_reps):
                        nc.vector.tensor_tensor(oh16, v16r, iota16, op.is_equal)

                oh16v = (oh16.rearrange("p v c -> p (v c)").rearrange(
                    "p (c v) -> p c v", v=16) if oh16_ls else None)
                # paired hist matmuls: frame f_local = 4*kq + g
                for kq in range(4):
                    khalf = (4 * ft + kq) % 8
                    for g in range(4):
                        fl = 4 * kq + g
                        for c2 in range(4):
                            cc0 = 8 * fl + 2 * c2
                            if oh16_ls:
                                lhsT = oh16v[:, cc0:cc0 + 2, :]
                            else:
                                lhsT = oh16[:, :, cc0:cc0 + 2].rearrange(
                                    "p v a -> p a v")
                            rhs = oh32[:, :, cc0:cc0 + 2].rearrange(
                                "p v a -> p a v")
                            nc.tensor.matmul(
                                psum_h[32 * g:32 * g + 32,
                                       64 * khalf:64 * khalf + 64],
                                lhsT=lhsT, rhs=rhs,
                                start=(c2 == 0), stop=(c2 == 3),
                                tile_position=(0, 32 * g),
                            )
            # Defer this half's evacuation until after the NEXT half's hist
            # matmuls are emitted: a waiting evac instruction at a FIFO queue
            # head would otherwise stall the next iterations' work behind it.
            def make_evac(fill, half, psum_h):
                def run():
                    # parity-deinterleaved evacuation: hstP[pr][:, 32*k + v32]
                    # so B-matmul stationary slices are contiguous (1 free dim)
                    psv = psum_h.rearrange("p (k r v) -> p k r v", k=8, r=2)
                    hstP = []
                    for pr in range(2):
                        h_ = hsts.tile([128, 256], dt.bfloat16, tag=f"hstP{pr}")
                        if pr == 0:
                            nc.vector.tensor_copy(out=h_, in_=psv[:, :, pr, :])
                        else:
                            nc.scalar.activation(h_, psv[:, :, pr, :], act.Copy)
                        hstP.append(h_)
                    # pbt[64*(kg%2) + 32*kt + v32, 64*(kg//2) + 16g + v16]
                    #  = histT(frame k = 8*half + 2*kg + kt); so row-block
                    # 32*kq holds frames kq and kq+4 side by side in cols
                    pbt = ps_bt.tile([128, 128], dt.float32, tag="pbt")
                    for kg in range(4):
                        rowbase = 64 * (kg % 2)
                        colbase = 64 * (kg // 2)
                        for pr in range(2):
                            nc.tensor.matmul(
                                pbt[rowbase:rowbase + 64,
                                    colbase:colbase + 64],
                                lhsT=hstP[pr][:, 64 * kg:64 * kg + 64],
                                rhs=bsel[pr],
                                start=(pr == 0), stop=(pr == 1),
                                tile_position=(0, rowbase),
                            )
                    for kq in range(4):
                        # frames kA = 8*half + kq and kA+4 -> x2 col runs 256
                        # apart; one copy with a [j=2, 64] destination AP
                        kA = 8 * half + kq
                        f0 = 16 * (64 * fill + 16 * (kA // 4) + 4 * kq)
                        dbase = x2[32 * kq:32 * kq + 32, f0:f0 + 64]
                        dst = bass.AP(
                            tensor=dbase.tensor, offset=dbase.offset,
                            ap=[dbase.ap[0], [256, 2], [1, 64]],
                        )
                        nc.scalar.activation(
                            dst, pbt[32 * kq:32 * kq + 32, :].rearrange(
                                "p (j n) -> p j n", j=2),
                            act.Copy)
                return run

            # mirrors run two halves after their fill's evac2 so they never
            # wait at the head of the sync queue
            hidx = 2 * fill + half
            while mirror_q and mirror_q[0][0] <= hidx:
                mirror_q.pop(0)[1]()
            if pending_evac is not None:
                erun, ef, eh = pending_evac
                erun()
                if eh == 1:
                    mirror_q.append((2 * ef + 4, make_mirror(ef)))
            pending_evac = (make_evac(fill, half, psum_h), fill, half)
            # interleave sims / windows / fc emission for completed blocks
            if half == 1:
                if fill >= 5 and (fill - 5) % 2 == 0:
                    emit_block_sims((fill - 5) // 2)
                if fill >= 6 and (fill - 6) % 2 == 0:
                    emit_wins((fill - 6) // 2)
                if phases != "nofc" and fill >= 10 and (fill - 10) % 2 == 0:
                    emit_fc((fill - 10) // 2)
        if pending_evac is not None:
            erun, ef, eh = pending_evac
            erun()
            pending_evac = None
            if eh == 1:
                mirror_q.append((0, make_mirror(ef)))
        for _, mrun in mirror_q:
            mrun()
        mirror_q = []
        for i in range(6, NBLK):
            emit_block_sims(i)
        for i in range(5, NBLK):
            emit_wins(i)
        if phases == "nofc":
            for i in range(NBLK):
                nc.sync.dma_start(out=y_d[128 * i:128 * i + 128, :], in_=zeros128)
            fctx.close()
            sctx.close()
            hctx.close()
            return nc
        for i in range(3, NBLK):
            emit_fc(i)

        fctx.close()
        sctx.close()
        hctx.close()

    return nc


def _host_inputs(frames, W_fc, b_fc):
    wfct = np.ascontiguousarray(np.asarray(W_fc, np.float32).T)
    bfc = np.ascontiguousarray(np.asarray(b_fc, np.float32)[None, :])
    f = np.asarray(frames)
    return [
        {
            "frames": np.ascontiguousarray(f[i].reshape(f.shape[1], PX * 3), dtype=np.int32),
            "wfct": wfct,
            "bfc": bfc,
        }
        for i in range(f.shape[0])
    ]


def kernel(frames, W_fc, b_fc):
    from concourse.bass_utils import run_bass_kernel_spmd

    if "nc" not in _CACHE:
        nc = build_nc()
        nc.finalize()
        _CACHE["nc"] = nc
    nc = _CACHE["nc"]
    in_maps = _host_inputs(frames, W_fc, b_fc)
    res = run_bass_kernel_spmd(nc, in_maps, list(range(B)))
    out = np.stack([res.results[i]["y"] for i in range(B)], axis=0)
    return out.astype(np.float32)



# revision 35
# speedup vs baseline: 1.1288x; 1.1288x over previous
"""Trainium2 Bass kernel for nn_ColorHistograms (histogram -> banded sims -> windowed fc).

Sharding: data-parallel, one video (T=1024 frames) per NeuronCore, 8 cores, SPMD.

Per-core pipeline:
  1. 512-bin color histogram per frame, factored 512 = 32 x 16:
     DVE builds one-hots (is_equal vs iota consts, bf16 2x mode); PE contracts
     128-pixel chunks (oh16^T @ oh32) into PSUM, 4 frames col-tiled concurrently.
  2. PE transposes + compaction copies -> X2[32*slab + v32, 16*F + v16] bf16
     (exact integer counts); slab-mirror DMAs so each 32-row slab has all frames.
  3. Banded sims (|t-s| <= 50 needs only adjacent 128-blocks): 16 K=32 matmuls
     per block pair accumulate exact integer <h_t, h_s> into PSUM strips.
  4. Strips -> DRAM; skewed-AP DMA (stride 385) gathers windows[t, w] =
     sims[t, t+w-50]. The w=50 column is ||h_t||^2 (diag) for free.
  5. Normalize after the gather: inv = rsqrt(diag) via sqrt + reciprocal +
     one Newton step; winN = win * inv[t] * inv[t+w-50] (zero-padded edges
     reproduce the reference's zero padding).
  6. fc: PE transpose of winN -> matmul with host-pretransposed W_fc^T;
     bias + relu on DVE; DMA out.
"""

import numpy as np

B = 8
T_FULL = 1024
PX = 32 * 32
LOOKUP_WINDOW = 101
OUT_DIM = 128

_CACHE = {}


def build_nc(T=T_FULL, quant_offset=None, phases="all", dve_reps=1, oh16_ls=True):
    # HW DVE float->int32 output conversion rounds to nearest; the sim
    # truncates. Midpoint offsets make every value >=1/128 away from a
    # rounding boundary, so any nearest-mode is exact on HW.
    # quant_offset=(0,0) reproduces the truncating sim semantics.
    from contextlib import ExitStack

    import concourse.bass as bass
    import concourse.mybir as mybir
    from concourse import bacc
    from concourse.masks import make_identity
    from concourse.tile import TileContext

    dt = mybir.dt
    op = mybir.AluOpType
    act = mybir.ActivationFunctionType
    qoff32, qoff64 = (-15.5 / 32.0, -31.5 / 64.0) if quant_offset is None else quant_offset

    NBLK = T // 128
    NFILL = T // 64

    nc = bacc.Bacc()
    frames_d = nc.declare_dram_parameter("frames", [T, PX * 3], dt.int32, isOutput=False)
    wfct_d = nc.declare_dram_parameter("wfct", [LOOKUP_WINDOW, OUT_DIM], dt.float32, isOutput=False)
    bfc_d = nc.declare_dram_parameter("bfc", [1, OUT_DIM], dt.float32, isOutput=False)
    y_d = nc.declare_dram_parameter("y", [T, OUT_DIM], dt.float32, isOutput=True)

    strips_d = nc.dram_tensor("strips", [NBLK, 128, 384], dt.float32)
    invpad_d = nc.dram_tensor("invpad", [1, 64 + T + 64], dt.float32)

    with TileContext(nc) as tc, ExitStack() as ctx:
        singles = ctx.enter_context(tc.tile_pool(name="singles", bufs=1))
        raws = ctx.enter_context(tc.tile_pool(name="raws", bufs=4))
        preps = ctx.enter_context(tc.tile_pool(name="preps", bufs=4))
        ohs = ctx.enter_context(tc.tile_pool(name="ohs", bufs=4))
        hsts = ctx.enter_context(tc.tile_pool(name="hsts", bufs=3))
        strips_p = ctx.enter_context(tc.tile_pool(name="strips_p", bufs=2))
        winp = ctx.enter_context(tc.tile_pool(name="winp", bufs=3))
        allwin = ctx.enter_context(tc.tile_pool(name="allwin", bufs=1))
        smalls = ctx.enter_context(tc.tile_pool(name="smalls", bufs=8))
        outs = ctx.enter_context(tc.tile_pool(name="outs", bufs=3))

        # ---------------- constants ----------------
        iota32 = singles.tile([128, 32, 128], dt.bfloat16)
        nc.gpsimd.iota(iota32, pattern=[[1, 32], [0, 128]], base=0,
                       channel_multiplier=0, allow_small_or_imprecise_dtypes=True)
        iota16 = singles.tile([128, 16, 128], dt.bfloat16)
        nc.gpsimd.iota(iota16, pattern=[[1, 16], [0, 128]], base=0,
                       channel_multiplier=0, allow_small_or_imprecise_dtypes=True)
        iotac16 = singles.tile([128, 8, 8], dt.int16)
        # value = 16 * (8*fh + c) : col base within a local_scatter half
        nc.gpsimd.iota(iotac16, pattern=[[128, 8], [16, 8]], base=0,
                       channel_multiplier=0)
        # dense variant over all 16 frames: value = 16*(8*(f%8) + c)
        iotacf = singles.tile([128, 2, 8, 8], dt.int16)
        nc.gpsimd.iota(iotacf, pattern=[[0, 2], [128, 8], [16, 8]], base=0,
                       channel_multiplier=0)
        ones_ls = singles.tile([128, 128], dt.bfloat16)
        nc.vector.memset(ones_ls, 1.0)
        ident_f = singles.tile([128, 128], dt.float32)
        make_identity(nc, ident_f)
        # parity selectors for the hist evacuation matmuls:
        # bsel[pr][32g + 16h + v16, 16g + v16] = 1 iff h == pr
        bsel = []
        for pr in range(2):
            tmpb = singles.tile([128, 4, 16], dt.int16, tag=f"tmpb{pr}")
            nc.gpsimd.iota(tmpb, pattern=[[-32, 4], [-1, 16]], base=-16 * pr,
                           channel_multiplier=1)
            bs = singles.tile([128, 64], dt.bfloat16, tag=f"bs{pr}")
            nc.vector.tensor_scalar(
                bs, tmpb.rearrange("p g v -> p (g v)"), 0.0, None, op0=op.is_equal)
            bsel.append(bs)
        wfct_sb = singles.tile([LOOKUP_WINDOW, OUT_DIM], dt.float32)
        nc.sync.dma_start(out=wfct_sb, in_=wfct_d[:, :])
        b_rep = singles.tile([128, OUT_DIM], dt.float32)
        nc.sync.dma_start(out=b_rep, in_=bfc_d[:, :].broadcast_to([128, OUT_DIM]))
        zeros128 = singles.tile([128, 128], dt.float32)
        nc.vector.memset(zeros128, 0.0)

        # zero the never-written edges of strips + invpad
        nc.sync.dma_start(out=strips_d[0, :, 0:128], in_=zeros128)
        nc.sync.dma_start(out=strips_d[NBLK - 1, :, 256:384], in_=zeros128)
        nc.sync.dma_start(out=invpad_d[0:1, 0:64], in_=zeros128[0:1, 0:64])
        nc.sync.dma_start(out=invpad_d[0:1, 64 + T:64 + T + 64], in_=zeros128[0:1, 0:64])

        # X2[32*slab + v32, 16*F + v16]  (bin = 16*v32 + v16)
        x2 = singles.tile([128, 16 * T], dt.bfloat16)

        frames_flat = frames_d[:, :].flatten()

        # ---------------- Phase H: histograms ----------------
        # Paired-chunk matmuls: each K-row carries the 2 pixels of chunks
        # (2c2, 2c2+1) via disjoint col-blocks, so one LDW+MM covers 256 px
        # (4 MM + 4 LDW per frame instead of 8+8). Real [16, 32] blocks land
        # at parity-split rows (h = chunk LSB); cross-parity blocks are
        # garbage. PSUM[32g + 16h + v16, 64k + 32pr + v32], 32 frames/bank.
        # Evacuate the whole bank with one dense DVE copy (garbage included),
        # then parity-pure strided LDWs x the constant selector bsel[pr]
        # (zeroes garbage rows, sums the two h-rows) accumulate histT in
        # PSUM; small copies scatter histT blocks into x2.
        hctx = ExitStack()
        ps_h = hctx.enter_context(tc.tile_pool(name="ps_h", bufs=2, space="PSUM"))
        ps_bt = hctx.enter_context(tc.tile_pool(name="ps_bt", bufs=2, space="PSUM"))
        sctx = ExitStack()
        ps_s = sctx.enter_context(tc.tile_pool(name="ps_s", bufs=2, space="PSUM"))
        x2v = x2.rearrange("p (f v) -> p f v", v=16)

        def emit_block_sims(i):
            js = [j for j in (i - 1, i, i + 1) if 0 <= j < NBLK]
            if phases == "diag":
                js = [i]
            # all pairs of block i contract on slab i%4 (mirrors only need to
            # populate 3 of the 4 slabs per block)
            rho = i % 4
            strip_sb = strips_p.tile([128, 384], dt.float32, tag="strip_sb")
            for j in js:
                jj = j - i + 1
                # one PSUM bank per pair: concurrent row-group matmul streams
                # to a shared bank fault the PSUM
                psum_s = ps_s.tile([128, 128], dt.float32, tag="psum_s")
                for m in range(16):
                    nc.tensor.matmul(
                        psum_s,
                        lhsT=x2v[32 * rho:32 * rho + 32, 128 * i:128 * i + 128, m],
                        rhs=x2v[32 * rho:32 * rho + 32, 128 * j:128 * j + 128, m],
                        start=(m == 0), stop=(m == 15),
                        tile_position=(32 * rho, 0),
                    )
                nc.scalar.activation(
                    strip_sb[:, 128 * jj:128 * jj + 128], psum_s, act.Copy,
                )
            jj0, jj1 = js[0] - i + 1, js[-1] - i + 1
            nc.scalar.dma_start(
                out=strips_d[i, :, 128 * jj0:128 * (jj1 + 1)],
                in_=strip_sb[:, 128 * jj0:128 * (jj1 + 1)],
            )

        def make_mirror(fill):
            def run():
                # mirror this fill's columns to the slabs that block
                # b = fill//2 is contracted on ({b-1, b, b+1} % 4) so banded
                # sims for finished 128-frame blocks can run during phase H
                b = fill // 2
                needed = {(b - 1) % 4, b % 4, (b + 1) % 4}
                fcols = x2[:, 1024 * fill:1024 * fill + 1024].rearrange(
                    "p (m q) -> p m q", q=256)
                for kap in range(4):
                    srcv = fcols[32 * kap:32 * kap + 32, :,
                                 64 * kap:64 * kap + 64]
                    for rho in sorted(needed - {kap}):
                        nc.sync.dma_start(
                            out=fcols[32 * rho:32 * rho + 32, :,
                                      64 * kap:64 * kap + 64],
                            in_=srcv)
            return run

        fctx = ExitStack()
        ps_f = fctx.enter_context(tc.tile_pool(name="ps_f", bufs=1, space="PSUM"))
        strips_flat = strips_d[:, :, :].flatten()
        invpad_flat = invpad_d[:, :].flatten()
        wins = {}
        invs = {}

        def emit_wins(i):
            # skewed-AP gather windows[t, w] = sims[t, t+w-50]; col 50 is
            # ||h_t||^2; inv = rsqrt via sqrt + reciprocal + one Newton step
            win = allwin.tile([128, LOOKUP_WINDOW], dt.float32, tag=f"win{i}")
            src = bass.AP(
                tensor=strips_flat.tensor,
                offset=strips_flat.offset + i * 128 * 384 + 78,
                ap=[[385, 128], [1, LOOKUP_WINDOW]],
            )
            nc.gpsimd.dma_start(out=win, in_=src)
            wins[i] = win
            nrm2 = smalls.tile([128, 1], dt.float32, tag=f"nrm2_{i}")
            nc.scalar.activation(nrm2, win[:, 50:51], act.Copy)
            s0 = smalls.tile([128, 1], dt.float32, tag=f"s0_{i}")
            nc.scalar.activation(s0, nrm2, act.Sqrt)
            b0 = smalls.tile([128, 1], dt.float32, tag=f"b0_{i}")
            nc.vector.reciprocal(b0, s0)
            c0 = smalls.tile([128, 1], dt.float32, tag=f"c0_{i}")
            nc.vector.tensor_tensor(c0, b0, b0, op.mult)
            d0 = smalls.tile([128, 1], dt.float32, tag=f"d0_{i}")
            nc.vector.tensor_tensor(d0, c0, nrm2, op.mult)
            e0 = smalls.tile([128, 1], dt.float32, tag=f"e0_{i}")
            nc.vector.tensor_scalar(e0, d0, -0.5, 1.5, op0=op.mult, op1=op.add)
            inv = smalls.tile([128, 1], dt.float32, tag=f"inv_{i}")
            nc.vector.tensor_tensor(inv, b0, e0, op.mult)
            invs[i] = inv
            dst = bass.AP(
                tensor=invpad_flat.tensor,
                offset=invpad_flat.offset + 64 + 128 * i,
                ap=[[1, 128], [1, 1]],
            )
            nc.gpsimd.dma_start(out=dst, in_=inv)

        def emit_fc(i):
            win = wins[i]
            invwin = winp.tile([128, LOOKUP_WINDOW], dt.float32, tag="invwin")
            src = bass.AP(
                tensor=invpad_flat.tensor,
                offset=invpad_flat.offset + 64 + 128 * i - 50,
                ap=[[1, 128], [1, LOOKUP_WINDOW]],
            )
            nc.gpsimd.dma_start(out=invwin, in_=src)
            wn1 = winp.tile([128, LOOKUP_WINDOW], dt.float32, tag="wn1")
            nc.vector.tensor_scalar(wn1, win, invs[i], None, op0=op.mult)
            winN = winp.tile([128, LOOKUP_WINDOW], dt.float32, tag="winN")
            nc.vector.tensor_tensor(winN, wn1, invwin, op.mult)
            ps_w = ps_f.tile([LOOKUP_WINDOW, 128], dt.float32, tag="psw")
            nc.tensor.transpose(ps_w, winN, ident_f)
            wT = winp.tile([LOOKUP_WINDOW, 128], dt.float32, tag="wT")
            nc.scalar.activation(wT, ps_w, act.Copy)
            ps_o = ps_f.tile([128, OUT_DIM], dt.float32, tag="pso")
            nc.tensor.matmul(ps_o, lhsT=wT, rhs=wfct_sb, start=True, stop=True)
            s2 = outs.tile([128, OUT_DIM], dt.float32, tag="s2")
            nc.vector.tensor_tensor(s2, ps_o, b_rep, op.add)
            yt = outs.tile([128, OUT_DIM], dt.float32, tag="yt")
            nc.vector.tensor_scalar_max(yt, s2, 0.0)
            nc.gpsimd.dma_start(out=y_d[128 * i:128 * i + 128, :], in_=yt)

        pending_evac = None
        mirror_q = []
        for fill in range(NFILL):
          for half in range(2):
            psum_h = ps_h.tile([128, 512], dt.float32, tag="ph")
            for ft2 in range(2):
                ft = 2 * half + ft2
                fbase = 64 * fill + 16 * ft
                raw = raws.tile([128, 16, 24], dt.int32, tag="raw")
                # flat = f*3072 + (8p + c)*3 + ch  ->  dest [p, f, (c, ch)]
                src = bass.AP(
                    tensor=frames_flat.tensor,
                    offset=frames_flat.offset + fbase * 3072,
                    ap=[[24, 128], [3072, 16], [1, 24]],
                )
                nc.sync.dma_start(out=raw, in_=src)

                # q = round/trunc(raw * (1/32) + off) ; quant_offset makes
                # either conversion mode exact. One dense quant op on ACT
                # frees the DVE for the one-hots; qg6 halves qg on the DVE
                # (round-nearest: k*0.5 - 0.25 rounds to floor(k/2)).
                q_all = preps.tile([128, 16, 24], dt.int32, tag="qall")
                nc.scalar.activation(q_all, raw, act.Copy,
                                     bias=qoff32, scale=1.0 / 32.0)
                qv = q_all.rearrange("p f (c ch) -> p f c ch", ch=3)
                qg6 = preps.tile([128, 16, 8], dt.int32, tag="qg6")
                nc.vector.tensor_scalar(qg6, qv[:, :, :, 1], 0.5, -0.25,
                                        op0=op.mult, op1=op.add)
                v32b = preps.tile([128, 16, 8], dt.bfloat16, tag="v32b")
                nc.vector.scalar_tensor_tensor(v32b, qv[:, :, :, 0], 4.0, qg6,
                                               op0=op.mult, op1=op.add)
                t1 = preps.tile([128, 16, 8], dt.int32, tag="t1")
                nc.vector.scalar_tensor_tensor(t1, qv[:, :, :, 1], 8.0,
                                               qv[:, :, :, 2],
                                               op0=op.mult, op1=op.add)
                v16b = preps.tile([128, 16, 8], dt.bfloat16, tag="v16b")
                if oh16_ls:
                    v16i = preps.tile([128, 16, 8], dt.int16, tag="v16i")
                    # idx = 16*cc + v16 for local_scatter one-hot build
                    nc.vector.scalar_tensor_tensor(v16i, qg6, -16.0, t1,
                                                   op0=op.mult, op1=op.add)
                    nc.vector.tensor_tensor(
                        v16i, v16i,
                        iotacf.rearrange("p h f c -> p (h f) c"),
                        op.add)
                else:
                    nc.vector.scalar_tensor_tensor(v16b, qg6, -16.0, t1,
                                                   op0=op.mult, op1=op.add)

                # one-hots [128, v, 128] ; px-col cc = 8*fl + c
                # (two halves so frames 0-7's matmuls can start earlier)
                oh32 = ohs.tile([128, 32, 128], dt.bfloat16, tag="oh32")
                v32r = v32b.rearrange("p f c -> p (f c)").unsqueeze(1).broadcast_to([128, 32, 128])
                oh16 = ohs.tile([128, 16, 128], dt.bfloat16, tag="oh16")
                for _rep in range(dve_reps):
                    nc.vector.tensor_tensor(oh32[:, :, 0:64], v32r[:, :, 0:64],
                                            iota32[:, :, 0:64], op.is_equal)
                    nc.vector.tensor_tensor(oh32[:, :, 64:128], v32r[:, :, 64:128],
                                            iota32[:, :, 64:128], op.is_equal)
                if oh16_ls:
                    # oh16[p, 16*cc + v] = 1 at v = v16 (cc-major layout)
                    oh16cm = oh16.rearrange("p v c -> p (v c)")
                    vflat = v16i.rearrange("p f c -> p (f c)")
                    for h in range(2):
                        nc.gpsimd.local_scatter(
                            oh16cm[:, 1024 * h:1024 * h + 1024],
                            ones_ls[:, 64 * h:64 * h + 64],
                            vflat[:, 64 * h:64 * h + 64],
                            channels=128, num_elems=1024, num_idxs=64,
                        )
                else:
                    v16r = v16b.rearrange("p f c -> p (f c)").unsqueeze(1).broadcast_to([128, 16, 128])
                    for _rep in range(dve_reps):
                        nc.vector.tensor_tensor(oh16, v16r, iota16, op.is_equal)

                oh16v = (oh16.rearrange("p v c -> p (v c)").rearrange(
                    "p (c v) -> p c v", v=16) if oh16_ls else None)
                # paired hist matmuls: frame f_local = 4*kq + g
                for kq in range(4):
                    khalf = (4 * ft + kq) % 8
                    for g in range(4):
                        fl = 4 * kq + g
                        for c2 in range(4):
                            cc0 = 8 * fl + 2 * c2
                            if oh16_ls:
                                lhsT = oh16v[:, cc0:cc0 + 2, :]
                            else:
                                lhsT = oh16[:, :, cc0:cc0 + 2].rearrange(
                                    "p v a -> p a v")
                            rhs = oh32[:, :, cc0:cc0 + 2].rearrange(
                                "p v a -> p a v")
                            nc.tensor.matmul(
                                psum_h[32 * g:32 * g + 32,
                                       64 * khalf:64 * khalf + 64],
                                lhsT=lhsT, rhs=rhs,
                                start=(c2 == 0), stop=(c2 == 3),
                                tile_position=(0, 32 * g),
                            )
            # Defer this half's evacuation until after the NEXT half's hist
            # matmuls are emitted: a waiting evac instruction at a FIFO queue
            # head would otherwise stall the next iterations' work behind it.
            def make_evac(fill, half, psum_h):
                def run():
                    # parity-deinterleaved evacuation: hstP[pr][:, 32*k + v32]
                    # so B-matmul stationary slices are contiguous (1 free dim)
                    psv = psum_h.rearrange("p (k r v) -> p k r v", k=8, r=2)
                    hstP = []
                    for pr in range(2):
                        h_ = hsts.tile([128, 256], dt.bfloat16, tag=f"hstP{pr}")
                        nc.scalar.activation(h_, psv[:, :, pr, :], act.Copy)
                        hstP.append(h_)
                    # pbt[64*(kg%2) + 32*kt + v32, 64*(kg//2) + 16g + v16]
                    #  = histT(frame k = 8*half + 2*kg + kt); so row-block
                    # 32*kq holds frames kq and kq+4 side by side in cols
                    pbt = ps_bt.tile([128, 128], dt.float32, tag="pbt")
                    for kg in range(4):
                        rowbase = 64 * (kg % 2)
                        colbase = 64 * (kg // 2)
                        for pr in range(2):
                            nc.tensor.matmul(
                                pbt[rowbase:rowbase + 64,
                                    colbase:colbase + 64],
                                lhsT=hstP[pr][:, 64 * kg:64 * kg + 64],
                                rhs=bsel[pr],
                                start=(pr == 0), stop=(pr == 1),
                                tile_position=(0, rowbase),
                            )
                    for kq in range(4):
                        # frames kA = 8*half + kq and kA+4 -> x2 col runs 256
                        # apart; one copy with a [j=2, 64] destination AP
                        kA = 8 * half + kq
                        f0 = 16 * (64 * fill + 16 * (kA // 4) + 4 * kq)
                        dbase = x2[32 * kq:32 * kq + 32, f0:f0 + 64]
                        dst = bass.AP(
                            tensor=dbase.tensor, offset=dbase.offset,
                            ap=[dbase.ap[0], [256, 2], [1, 64]],
                        )
                        nc.scalar.activation(
                            dst, pbt[32 * kq:32 * kq + 32, :].rearrange(
                                "p (j n) -> p j n", j=2),
                            act.Copy)
                return run

            # mirrors run two halves after their fill's evac2 so they never
            # wait at the head of the sync queue
            hidx = 2 * fill + half
            while mirror_q and mirror_q[0][0] <= hidx:
                mirror_q.pop(0)[1]()
            if pending_evac is not None:
                erun, ef, eh = pending_evac
                erun()
                if eh == 1:
                    mirror_q.append((2 * ef + 3, make_mirror(ef)))
            pending_evac = (make_evac(fill, half, psum_h), fill, half)
            # interleave sims / windows / fc emission for completed blocks
            if half == 1:
                if fill >= 4 and (fill - 4) % 2 == 0:
                    emit_block_sims((fill - 4) // 2)
                if fill >= 5 and (fill - 5) % 2 == 0:
                    emit_wins((fill - 5) // 2)
                if phases != "nofc" and fill >= 9 and (fill - 9) % 2 == 0:
                    emit_fc((fill - 9) // 2)
        if pending_evac is not None:
            erun, ef, eh = pending_evac
            erun()
            pending_evac = None
            if eh == 1:
                mirror_q.append((0, make_mirror(ef)))
        for _, mrun in mirror_q:
            mrun()
        mirror_q = []
        for i in range(6, NBLK):
            emit_block_sims(i)
        for i in range(6, NBLK):
            emit_wins(i)
        if phases == "nofc":
            for i in range(NBLK):
                nc.sync.dma_start(out=y_d[128 * i:128 * i + 128, :], in_=zeros128)
            fctx.close()
            sctx.close()
            hctx.close()
            return nc
        for i in range(4, NBLK):
            emit_fc(i)

        fctx.close()
        sctx.close()
        hctx.close()

    return nc


def _host_inputs(frames, W_fc, b_fc):
    wfct = np.ascontiguousarray(np.asarray(W_fc, np.float32).T)
    bfc = np.ascontiguousarray(np.asarray(b_fc, np.float32)[None, :])
    f = np.asarray(frames)
    return [
        {
            "frames": np.ascontiguousarray(f[i].reshape(f.shape[1], PX * 3), dtype=np.int32),
            "wfct": wfct,
            "bfc": bfc,
        }
        for i in range(f.shape[0])
    ]


def kernel(frames, W_fc, b_fc):
    from concourse.bass_utils import run_bass_kernel_spmd

    if "nc" not in _CACHE:
        nc = build_nc()
        nc.finalize()
        _CACHE["nc"] = nc
    nc = _CACHE["nc"]
    in_maps = _host_inputs(frames, W_fc, b_fc)
    res = run_bass_kernel_spmd(nc, in_maps, list(range(B)))
    out = np.stack([res.results[i]["y"] for i in range(B)], axis=0)
    return out.astype(np.float32)

